# revision 1
# baseline (speedup 1.0000x reference)
"""DiffusionNetAutoencoder (4x ChebConv, K=6) Trainium2 kernel on 8 NeuronCores.

Distribution: nodes dest-sharded across 8 cores (12500 each); each core owns
the edges into its nodes, sorted by destination. Each destination's edge run
is padded to a multiple of 4 and runs are packed into 128-edge chunks with no
run straddling a chunk (pad slots carry norm=0).

SpMM: indirect-DMA gather of source rows from a full fp32 DRAM table ->
DVE scale by (s*norm) -> one PE matmul per 2048-edge group against a
stationary "quartile prefix" matrix QTRI (per-chunk cumsum sampled at every
4th slot) -> quartile prefixes to DRAM -> tiny indirect gathers of per-dest
end/start prefixes -> block = ends - starts + prefill (forward/Clenshaw
update, bias, ReLU fused) -> AllGather blocks into the next gather table.

Layers with F_in > F_out (2 and 4) use Clenshaw's recurrence at width F_out
(32 / 4):  b_k = 2 L b_{k+1} - b_{k+2} + Y_k,  out = L b_1 - b_2 + Y_0,
with Y_k = h @ W[k], so the SpMM width never exceeds 32.

Dense matmuls keep activations transposed ([F, N_local], staged via DRAM);
everything on device is fp32 (PSUM accumulates fp32).
"""
import contextlib
import numpy as np

import bass_rust
import concourse.bass as bass
import concourse.tile as tile
import concourse.mybir as mybir
from concourse.bass_utils import run_bass_kernel_spmd

F32 = mybir.dt.float32
I32 = mybir.dt.int32
AO = mybir.AluOpType

N_NODES = 100000
K = 6
NC = 8
NLOC = N_NODES // NC          # 12500
NLOC_PAD = 12544              # 128 * 98
NCH = NLOC_PAD // 128         # 98 chunks of local dests
F_HID, F_LAT, F4 = 128, 32, 4


# --------------------------------------------------------------------------
# This container's walrus rejects >1 sync-wait per instruction; Tile attaches
# more. Split extras onto NoOp carriers (same engine, just before the inst).
def _split_waits(nc, max_waits=1):
    for f in nc.m.functions:
        for bb in f.blocks:
            new_insts = []
            for inst in bb.instructions:
                si = inst.sync_info
                if si is not None and si.on_wait and len(si.on_wait) > max_waits:
                    waits = list(si.on_wait)
                    extra, keep = waits[:-max_waits], waits[-max_waits:]
                    si.on_wait = keep
                    for j in range(0, len(extra), max_waits):
                        nop = bass_rust.InstNoOp(
                            name=nc.get_next_instruction_name(),
                            opcode="NoOp",
                            engine=inst.engine,
                            debug=inst.debug,
                            ins=[], outs=[],
                            descendants=bass_rust.InstructionNameOrderedSet(),
                            sync_info=mybir.SyncInfo(
                                on_wait=extra[j:j + max_waits], on_update=[]),
                            bass_nofuse=True,
                            text_hint=None,
                        )
                        nc.register_instruction(nop, overwrite=True)
                        new_insts.append(nop)
                new_insts.append(inst)
            bb.instructions[:] = new_insts


# --------------------------------------------------------------------------
def _host_prep(x, edge_index, laplacian):
    row = np.asarray(edge_index[0], np.int64)
    col = np.asarray(edge_index[1], np.int64)
    lap = np.asarray(laplacian, np.float32)

    w = np.where(row == col, np.float32(0), lap).astype(np.float32)
    deg = np.zeros(N_NODES, np.float32)
    np.add.at(deg, row, w)
    dis = np.where(deg > 0, 1.0 / np.sqrt(np.maximum(deg, np.float32(1e-30))), 0.0)
    dis = dis.astype(np.float32)
    norm = (-dis[row] * w * dis[col]).astype(np.float32)

    owner = row // NLOC
    per_core = []
    for p in range(NC):
        sel = np.nonzero(owner == p)[0]
        r_loc = (row[sel] - p * NLOC).astype(np.int64)
        order = np.argsort(r_loc, kind="stable")
        sel = sel[order]
        r_loc = r_loc[order]
        c_e = col[sel].astype(np.int32)
        n_e = norm[sel]

        d = np.bincount(r_loc, minlength=NLOC).astype(np.int64)
        assert d.max() <= 128, f"in-degree too large for one chunk: {d.max()}"
        pl = ((d + 3) // 4) * 4

        seg_start = np.zeros(NLOC, np.int64)
        pos = 0
        for i in range(NLOC):
            pli = pl[i]
            if pli == 0:
                seg_start[i] = pos
                continue
            room = 128 - (pos & 127)
            if pli > room:
                pos += room
            seg_start[i] = pos
            pos += pli

        first_edge = np.zeros(NLOC, np.int64)
        first_edge[1:] = np.cumsum(d)[:-1]
        rank = np.arange(len(sel), dtype=np.int64) - first_edge[r_loc]
        slot = seg_start[r_loc] + rank
        per_core.append(dict(c_e=c_e, n_e=n_e, slot=slot, d=d,
                             seg_start=seg_start, pl=pl, e_used=pos))

    e_pad = max(pc["e_used"] for pc in per_core)
    G = (e_pad + 2047) // 2048
    e_pad = G * 2048
    zq = e_pad // 4

    ins = []
    for p in range(NC):
        pc = per_core[p]
        cols = np.zeros(e_pad, np.int32)
        nrms = np.zeros(e_pad, np.float32)
        cols[pc["slot"]] = pc["c_e"]
        nrms[pc["slot"]] = pc["n_e"]
        colt = cols.reshape(G * 16, 128).T.copy()
        nrmt = nrms.reshape(G * 16, 128).T.copy()

        d, seg_start, pl = pc["d"], pc["seg_start"], pc["pl"]
        ends = np.full(NLOC_PAD, zq, np.int64)
        starts = np.full(NLOC_PAD, zq, np.int64)
        has = d > 0
        e_slot = seg_start + pl - 1
        ends[:NLOC][has] = e_slot[has] // 4
        s_ok = has & ((seg_start & 127) != 0)
        starts[:NLOC][s_ok] = (seg_start[s_ok] - 1) // 4
        endt = ends.reshape(NCH, 128).T.astype(np.int32).copy()
        stat = starts.reshape(NCH, 128).T.astype(np.int32).copy()

        xb = np.zeros((NLOC, F4), np.float32)
        xb[:, :3] = np.asarray(x[p * NLOC:(p + 1) * NLOC], np.float32)
        # cs-shift optimization: cs[i] = ce[i-1] * mask, valid iff no empty
        # dests among 0..NLOC-1 (Poisson(16) -> ~never). mask=0 where starts
        # would be the zero row (chunk-initial segment or padding dest).
        msk = np.zeros(NLOC_PAD, np.float32)
        msk[:NLOC][has & ((seg_start & 127) != 0)] = 1.0
        mskt = msk.reshape(NCH, 128).T.copy()
        ins.append(dict(colidx=colt, nrm=nrmt, ends=endt, starts=stat, xblk=xb,
                        csmask=mskt, all_nonzero=bool((d > 0).all())))
    return ins, G


def _weights_prep(W1, W2, W3, W4, b1, b2, b3, b4):
    W1 = np.asarray(W1, np.float32); W2 = np.asarray(W2, np.float32)
    W3 = np.asarray(W3, np.float32); W4 = np.asarray(W4, np.float32)
    w1s = np.zeros((128, F_HID), np.float32)
    for k in range(K):
        w1s[k * 4:k * 4 + 3, :] = W1[k]
    w2s = W2.transpose(1, 0, 2).reshape(F_HID, K * F_LAT)
    w3hi = W3[:4].reshape(4 * F_LAT, F_HID)
    w3lo = np.zeros((128, F_HID), np.float32)
    w3lo[:64] = W3[4:].reshape(2 * F_LAT, F_HID)
    w4s = np.zeros((F_HID, K * F4), np.float32)
    for k in range(K):
        w4s[:, k * 4:k * 4 + 3] = W4[k]
    qt1 = np.zeros((128, 32), np.float32)
    for j in range(32):
        qt1[: 4 * j + 4, j] = 1.0
    qt2 = 2.0 * qt1
    ident = np.eye(128, dtype=np.float32)
    bc1 = np.zeros((128, 1), np.float32); bc1[:, 0] = np.asarray(b1, np.float32)
    br2 = np.zeros((128, F_LAT), np.float32); br2[:] = np.asarray(b2, np.float32)
    bc3 = np.zeros((128, 1), np.float32); bc3[:, 0] = np.asarray(b3, np.float32)
    b4p = np.zeros(F4, np.float32); b4p[:3] = np.asarray(b4, np.float32)
    br4 = np.zeros((128, F4), np.float32); br4[:] = b4p
    parts = [("w1s", w1s), ("w2sA", w2s[:, :128]), ("w2sB", np.pad(w2s[:, 128:], ((0, 0), (0, 0)))),
             ("w3hi", w3hi), ("w3lo", w3lo), ("w4s", w4s), ("qt1", qt1),
             ("qt2", qt2), ("ident", ident), ("bc1", bc1), ("br2", br2),
             ("bc3", bc3), ("br4", br4)]
    offs, cur = {}, 0
    for nm, a in parts:
        assert a.shape[0] <= 128
        offs[nm] = (cur, a.shape[0], a.shape[1])
        cur += a.shape[1]
    wpack = np.zeros((128, cur), np.float32)
    for nm, a in parts:
        c0, r, c = offs[nm]
        wpack[:r, c0:c0 + c] = a
    return wpack, offs


# --------------------------------------------------------------------------
def build_program(G, offs, wtot, use_shift=False):
    EP = G * 2048
    QROWS = EP // 4 + 1
    nc = bass.Bass(trn_type="TRN2")

    colidx_i = nc.dram_tensor("colidx", [128, G * 16], I32, kind="ExternalInput")
    nrm_i = nc.dram_tensor("nrm", [128, G * 16], F32, kind="ExternalInput")
    ends_i = nc.dram_tensor("ends", [128, NCH], I32, kind="ExternalInput")
    starts_i = nc.dram_tensor("starts", [128, NCH], I32, kind="ExternalInput")
    csmask_i = nc.dram_tensor("csmask", [128, NCH], F32, kind="ExternalInput")
    xblk_i = nc.dram_tensor("xblk", [NLOC, F4], F32, kind="ExternalInput")
    wpack_i = nc.dram_tensor("wpack", [128, wtot], F32, kind="ExternalInput")
    out_o = nc.dram_tensor("out", [NLOC, 3], F32, kind="ExternalOutput")

    t4a = nc.dram_tensor("t4a", [N_NODES, F4], F32, addr_space="Shared")
    t4b = nc.dram_tensor("t4b", [N_NODES, F4], F32, addr_space="Shared")
    t32a = nc.dram_tensor("t32a", [N_NODES, F_LAT], F32, addr_space="Shared")
    t32b = nc.dram_tensor("t32b", [N_NODES, F_LAT], F32, addr_space="Shared")
    q4 = nc.dram_tensor("q4", [QROWS, F4], F32)
    q32 = nc.dram_tensor("q32", [QROWS, F_LAT], F32)
    blk4 = nc.dram_tensor("blk4", [NLOC, F4], F32)
    blk32 = nc.dram_tensor("blk32", [NLOC, F_LAT], F32)
    ybuf2 = nc.dram_tensor("ybuf2", [NLOC_PAD, K * F_LAT], F32)
    ybuf4 = nc.dram_tensor("ybuf4", [NLOC_PAD, K * F4], F32)
    h1T_d = nc.dram_tensor("h1T_d", [F_HID, NLOC_PAD], F32)
    h3T_d = nc.dram_tensor("h3T_d", [F_HID, NLOC_PAD], F32)
    thi_d = nc.dram_tensor("thi_d", [128, NLOC_PAD], F32)
    tlo_d = nc.dram_tensor("tlo_d", [64, NLOC_PAD], F32)

    RG = [list(range(NC))]

    with tile.TileContext(nc) as tc, contextlib.ExitStack() as ctx:
        sbc = ctx.enter_context(tc.tile_pool(name="sbc", bufs=1))
        big = ctx.enter_context(tc.tile_pool(name="big", bufs=1))
        gb = ctx.enter_context(tc.tile_pool(name="gb", bufs=3))
        ykp = ctx.enter_context(tc.tile_pool(name="ykp", bufs=2))
        bd = ctx.enter_context(tc.tile_pool(name="bd", bufs=1))
        dtp = ctx.enter_context(tc.tile_pool(name="dtp", bufs=2))
        kpp = ctx.enter_context(tc.tile_pool(name="kpp", bufs=2))
        st = ctx.enter_context(tc.tile_pool(name="st", bufs=2))
        ps = ctx.enter_context(tc.tile_pool(name="ps", bufs=4, space="PSUM"))
        ps2 = ctx.enter_context(tc.tile_pool(name="ps2", bufs=2, space="PSUM"))

        # ---- constants
        colidx = sbc.tile([128, G * 16], I32)
        nc.sync.dma_start(colidx[:], colidx_i[:])
        nrm1 = sbc.tile([128, G * 16], F32)
        nc.sync.dma_start(nrm1[:], nrm_i[:])
        ends_sb = sbc.tile([128, NCH], I32)
        nc.sync.dma_start(ends_sb[:], ends_i[:])
        starts_sb = sbc.tile([128, NCH], I32)
        nc.sync.dma_start(starts_sb[:], starts_i[:])
        csmask = sbc.tile([128, NCH], F32)
        nc.sync.dma_start(csmask[:], csmask_i[:])
        wpack = sbc.tile([128, wtot], F32)
        nc.sync.dma_start(wpack[:], wpack_i[:])
        wt = {nm: wpack[:r, c0:c0 + c] for nm, (c0, r, c) in offs.items()}

        zt = st.tile([128, F_LAT], F32, tag="zt")
        nc.vector.memset(zt[:], 0.0)
        nc.sync.dma_start(q4[QROWS - 1:QROWS, :], zt[:1, :F4])
        nc.sync.dma_start(q32[QROWS - 1:QROWS, :], zt[:1, :F_LAT])

        def r3(t, f):
            return t[:].rearrange("p (m f) -> p m f", f=f)

        def tile_to_blk(t, dram, f):
            nc.sync.dma_start(
                dram[: 97 * 128, :].rearrange("(m q) f -> q m f", q=128),
                r3(t, f)[:, :97, :],
            )
            nc.sync.dma_start(dram[97 * 128:NLOC, :], r3(t, f)[:84, 97, :])

        def ag(blk, table):
            nc.gpsimd.collective_compute(
                "AllGather", AO.bypass, replica_groups=RG,
                ins=[blk.ap().opt()], outs=[table.ap().opt()])

        # x block -> x table
        x_loc = big.tile([128, NCH * F4], F32, tag="xloc")
        nc.vector.memset(r3(x_loc, F4)[:, 97, :], 0.0)
        nc.sync.dma_start(
            r3(x_loc, F4)[:, :97, :],
            xblk_i[: 97 * 128, :].rearrange("(m q) f -> q m f", q=128))
        nc.sync.dma_start(r3(x_loc, F4)[:84, 97, :], xblk_i[97 * 128:NLOC, :])
        tile_to_blk(x_loc, blk4, F4)
        ag(blk4, t4a)

        # ---- spmm
        def spmm(src_table, qbuf, f, scale2):
            qtri = wt["qt2"] if scale2 else wt["qt1"]
            for g in range(G):
                gt = gb.tile([128, 16 * f], F32, tag="gt")
                for m in range(16):
                    nc.gpsimd.indirect_dma_start(
                        out=gt[:, m * f:(m + 1) * f], out_offset=None,
                        in_=src_table[:],
                        in_offset=bass.IndirectOffsetOnAxis(
                            ap=colidx[:, g * 16 + m:g * 16 + m + 1], axis=0))
                vt = gb.tile([128, 16 * f], F32, tag="vt")
                nc.vector.tensor_tensor(
                    out=r3(vt, f), in0=r3(gt, f),
                    in1=nrm1[:, g * 16:(g + 1) * 16][:, :, None].to_broadcast(
                        [128, 16, f]),
                    op=AO.mult)
                pst = ps.tile([32, 16 * f], F32, tag="mm")
                nc.tensor.matmul(pst[:], lhsT=qtri, rhs=vt[:],
                                 start=True, stop=True)
                qs = gb.tile([32, 16 * f], F32, tag="qs")
                nc.vector.tensor_copy(out=qs[:], in_=pst[:])
                nc.sync.dma_start(
                    qbuf[g * 512:(g + 1) * 512, :].rearrange(
                        "(m j) f -> j m f", j=32),
                    r3(qs, f))
            ce = bd.tile([128, NCH * 32], F32, tag="ce")
            cs = bd.tile([128, NCH * 32], F32, tag="cs")
            for c in range(NCH):
                nc.gpsimd.indirect_dma_start(
                    out=ce[:, c * f:(c + 1) * f], out_offset=None, in_=qbuf[:],
                    in_offset=bass.IndirectOffsetOnAxis(
                        ap=ends_sb[:, c:c + 1], axis=0))
                if not use_shift:
                    nc.gpsimd.indirect_dma_start(
                        out=cs[:, c * f:(c + 1) * f], out_offset=None, in_=qbuf[:],
                        in_offset=bass.IndirectOffsetOnAxis(
                            ap=starts_sb[:, c:c + 1], axis=0))
            if use_shift:
                # cs[q, m] = ce[q-1, m] (q>0); cs[0, m] = ce[127, m-1]; cs[0,0]=0
                # then mask out chunk-initial/padding dests.
                nc.scalar.dma_start(cs[1:128, : NCH * f], ce[0:127, : NCH * f])
                nc.scalar.dma_start(cs[0:1, f: NCH * f],
                                    ce[127:128, : (NCH - 1) * f])
                nc.vector.memset(cs[0:1, :f], 0.0)
                nc.vector.tensor_tensor(
                    out=cs[:, : NCH * f].rearrange("p (m f) -> p m f", f=f),
                    in0=cs[:, : NCH * f].rearrange("p (m f) -> p m f", f=f),
                    in1=csmask[:, :, None].to_broadcast([128, NCH, f]),
                    op=AO.mult)
            dt = dtp.tile([128, NCH * 32], F32, tag="dt")
            nc.vector.tensor_tensor(out=dt[:, : NCH * f], in0=ce[:, : NCH * f],
                                    in1=cs[:, : NCH * f], op=AO.subtract)
            return dt

        def relu(ap):
            nc.vector.tensor_scalar_max(ap, ap, 0.0)

        def dcut(dt, f):
            return dt[:, : NCH * f]

        def rd3(dt, f):
            return dt[:, : NCH * f].rearrange("p (m f) -> p m f", f=f)

        # ============== Layer 1: forward, width 4, input x ==============
        txrm = big.tile([128, NCH * K * F4], F32, tag="txrm")

        def txrm_k(kk):
            return txrm[:].rearrange("p (m k f) -> p m k f", k=K, f=F4)[:, :, kk, :]

        nc.vector.tensor_copy(out=txrm_k(0), in_=r3(x_loc, F4))
        tabs = [t4a, t4b]
        for k in range(1, K):
            dt = spmm(tabs[(k + 1) % 2], q4, F4, scale2=(k > 1))
            if k > 1:
                nc.vector.tensor_tensor(out=rd3(dt, F4), in0=rd3(dt, F4),
                                        in1=txrm_k(k - 2), op=AO.subtract)
            nc.vector.tensor_copy(out=txrm_k(k), in_=rd3(dt, F4))
            if k < K - 1:
                tile_to_blk(dcut(dt, F4), blk4, F4)
                ag(blk4, tabs[k % 2])

        # dense L1: h1T = relu(w1s.T @ TxT + b1), staged to DRAM
        for c0 in range(0, NCH, 4):
            cn = min(4, NCH - c0)
            ho = st.tile([128, 4 * 128], F32, tag="ho")
            for c in range(c0, c0 + cn):
                ptr = ps2.tile([24, 128], F32, tag="tp")
                nc.tensor.transpose(ptr[:], txrm[:, c * 24:(c + 1) * 24],
                                    wt["ident"])
                t1t = st.tile([24, 128], F32, tag="sa")
                nc.vector.tensor_copy(out=t1t[:], in_=ptr[:])
                po = ps.tile([128, 128], F32, tag="mm")
                nc.tensor.matmul(po[:], lhsT=wt["w1s"][:24, :], rhs=t1t[:],
                                 start=True, stop=True)
                nc.vector.tensor_tensor(
                    out=ho[:, (c - c0) * 128:(c - c0 + 1) * 128], in0=po[:],
                    in1=wt["bc1"].to_broadcast([128, 128]), op=AO.add)
            relu(ho[:, : cn * 128])
            nc.sync.dma_start(h1T_d[:, c0 * 128:(c0 + cn) * 128], ho[:, : cn * 128])

        # ============== Layer 2: Clenshaw, width 32 ==============
        for c0 in range(0, NCH, 4):
            cn = min(4, NCH - c0)
            yst = st.tile([128, 4 * K * F_LAT], F32, tag="yst")
            for c in range(c0, c0 + cn):
                ht = st.tile([128, 128], F32, tag="ht")
                nc.scalar.dma_start(ht[:], h1T_d[:, c * 128:(c + 1) * 128])
                pa = ps.tile([128, 128], F32, tag="mm")
                nc.tensor.matmul(pa[:], lhsT=wt["w2sA"], rhs=ht[:],
                                 start=True, stop=True)
                ya = st.tile([128, 128], F32, tag="sa")
                nc.vector.tensor_copy(out=ya[:], in_=pa[:])
                pb = ps.tile([64, 128], F32, tag="mm")
                nc.tensor.matmul(pb[:], lhsT=wt["w2sB"], rhs=ht[:],
                                 start=True, stop=True)
                yb = st.tile([64, 128], F32, tag="sa")
                nc.vector.tensor_copy(out=yb[:], in_=pb[:])
                pt = ps2.tile([128, 128], F32, tag="tp")
                nc.tensor.transpose(pt[:], ya[:], wt["ident"])
                nc.vector.tensor_copy(
                    out=yst[:, (c - c0) * 192:(c - c0) * 192 + 128], in_=pt[:])
                pt2 = ps2.tile([128, 64], F32, tag="tp")
                nc.tensor.transpose(pt2[:], yb[:], wt["ident"][:64, :64])
                nc.vector.tensor_copy(
                    out=yst[:, (c - c0) * 192 + 128:(c - c0 + 1) * 192], in_=pt2[:])
            nc.sync.dma_start(
                ybuf2[c0 * 128:(c0 + cn) * 128, :].rearrange(
                    "(m q) f -> q m f", q=128),
                yst[:].rearrange("p (m f) -> p m f", f=K * F_LAT)[:, :cn, :])

        def clenshaw2(f, ybuf, tabs2, qbuf, blk, bias_row, last_to):
            ydram = ybuf.ap().rearrange("n (k f) -> n k f", f=f)

            def yk_load(kk):
                t = ykp.tile([128, NCH * 32], F32, tag="yk")
                nc.scalar.dma_start(
                    t[:, : NCH * f].rearrange("p (m f) -> p m f", f=f),
                    ydram[:, kk, :].rearrange("(m q) f -> q m f", q=128))
                return t

            y5 = yk_load(K - 1)
            k5 = kpp.tile([128, NCH * 32], F32, tag="kc")
            nc.vector.tensor_copy(out=k5[:, : NCH * f], in_=y5[:, : NCH * f])
            tile_to_blk(k5[:, : NCH * f], blk, f)
            ag(blk, tabs2[0])
            bk2_loc, bk1_loc = None, k5
            cur = 0
            for k in range(K - 2, -1, -1):
                dt = spmm(tabs2[cur], qbuf, f, scale2=(k > 0))
                yk = yk_load(k)
                if bk2_loc is not None:
                    nc.vector.tensor_tensor(out=yk[:, : NCH * f],
                                            in0=yk[:, : NCH * f],
                                            in1=bk2_loc[:, : NCH * f],
                                            op=AO.subtract)
                nc.vector.tensor_tensor(out=dcut(dt, f), in0=dcut(dt, f),
                                        in1=yk[:, : NCH * f], op=AO.add)
                if k == 0:
                    nc.vector.tensor_tensor(
                        out=rd3(dt, f), in0=rd3(dt, f),
                        in1=bias_row[:, None, :].to_broadcast([128, NCH, f]),
                        op=AO.add)
                    relu(dcut(dt, f))
                    if last_to is not None:
                        tile_to_blk(dcut(dt, f), blk, f)
                        ag(blk, last_to)
                    return dt
                kc = kpp.tile([128, NCH * 32], F32, tag="kc")
                nc.vector.tensor_copy(out=kc[:, : NCH * f], in_=dcut(dt, f))
                tile_to_blk(kc[:, : NCH * f], blk, f)
                ag(blk, tabs2[1 - cur])
                bk2_loc, bk1_loc = bk1_loc, kc
                cur = 1 - cur

        h2 = clenshaw2(F_LAT, ybuf2, [t32a, t32b], q32, blk32, wt["br2"],
                       last_to=t32a)

        # ============== Layer 3: forward, width 32 ==============
        def tx_to_T(src_ap, kk):
            dst = thi_d if kk < 4 else tlo_d
            r0 = (kk % 4) * 32 if kk < 4 else (kk - 4) * 32
            for c0 in range(0, NCH, 4):
                cn = min(4, NCH - c0)
                ts = st.tile([32, 4 * 128], F32, tag="ts")
                for c in range(c0, c0 + cn):
                    pt = ps2.tile([32, 128], F32, tag="tp")
                    nc.tensor.transpose(pt[:], src_ap[:, c * 32:(c + 1) * 32],
                                        wt["ident"])
                    nc.vector.tensor_copy(
                        out=ts[:, (c - c0) * 128:(c - c0 + 1) * 128], in_=pt[:])
                nc.sync.dma_start(dst[r0:r0 + 32, c0 * 128:(c0 + cn) * 128],
                                  ts[:, : cn * 128])

        tx_to_T(dcut(h2, F_LAT), 0)
        h2k = kpp.tile([128, NCH * 32], F32, tag="kc")
        nc.vector.tensor_copy(out=h2k[:], in_=h2[:])
        prev2, prev1 = None, h2k
        tabs3 = [t32a, t32b]
        cur = 0
        for k in range(1, K):
            dt = spmm(tabs3[cur], q32, F_LAT, scale2=(k > 1))
            if k > 1:
                nc.vector.tensor_tensor(out=dcut(dt, F_LAT), in0=dcut(dt, F_LAT),
                                        in1=prev2[:, : NCH * F_LAT],
                                        op=AO.subtract)
            tx_to_T(dcut(dt, F_LAT), k)
            if k < K - 1:
                tile_to_blk(dcut(dt, F_LAT), blk32, F_LAT)
                ag(blk32, tabs3[1 - cur])
                cur = 1 - cur
            kc = kpp.tile([128, NCH * 32], F32, tag="kc")
            nc.vector.tensor_copy(out=kc[:], in_=dt[:])
            prev2, prev1 = prev1, kc

        # dense L3: h3T = relu(w3hi.T @ thi + w3lo.T @ tlo + b3)
        for c0 in range(0, NCH, 4):
            cn = min(4, NCH - c0)
            ho = st.tile([128, 4 * 128], F32, tag="ho")
            for c in range(c0, c0 + cn):
                ti = st.tile([128, 128], F32, tag="ht")
                nc.scalar.dma_start(ti[:], thi_d[:, c * 128:(c + 1) * 128])
                tl = st.tile([64, 128], F32, tag="tl")
                nc.scalar.dma_start(tl[:], tlo_d[:64, c * 128:(c + 1) * 128])
                po = ps.tile([128, 128], F32, tag="mm")
                nc.tensor.matmul(po[:], lhsT=wt["w3hi"], rhs=ti[:],
                                 start=True, stop=False)
                nc.tensor.matmul(po[:], lhsT=wt["w3lo"][:64, :], rhs=tl[:],
                                 start=False, stop=True)
                nc.vector.tensor_tensor(
                    out=ho[:, (c - c0) * 128:(c - c0 + 1) * 128], in0=po[:],
                    in1=wt["bc3"].to_broadcast([128, 128]), op=AO.add)
            relu(ho[:, : cn * 128])
            nc.sync.dma_start(h3T_d[:, c0 * 128:(c0 + cn) * 128], ho[:, : cn * 128])

        # ============== Layer 4: Clenshaw, width 4 ==============
        for c0 in range(0, NCH, 4):
            cn = min(4, NCH - c0)
            yst = st.tile([128, 4 * K * F4], F32, tag="yst4")
            for c in range(c0, c0 + cn):
                ht = st.tile([128, 128], F32, tag="ht")
                nc.scalar.dma_start(ht[:], h3T_d[:, c * 128:(c + 1) * 128])
                pa = ps.tile([K * F4, 128], F32, tag="mm")
                nc.tensor.matmul(pa[:], lhsT=wt["w4s"], rhs=ht[:],
                                 start=True, stop=True)
                ya = st.tile([K * F4, 128], F32, tag="sa")
                nc.vector.tensor_copy(out=ya[:], in_=pa[:])
                pt = ps2.tile([128, K * F4], F32, tag="tp")
                nc.tensor.transpose(pt[:], ya[:], wt["ident"][: K * F4, : K * F4])
                nc.vector.tensor_copy(
                    out=yst[:, (c - c0) * K * F4:(c - c0 + 1) * K * F4], in_=pt[:])
            nc.sync.dma_start(
                ybuf4[c0 * 128:(c0 + cn) * 128, :].rearrange(
                    "(m q) f -> q m f", q=128),
                yst[:].rearrange("p (m f) -> p m f", f=K * F4)[:, :cn, :])

        out_blk = clenshaw2(F4, ybuf4, [t4a, t4b], q4, blk4, wt["br4"],
                            last_to=None)
        ob = rd3(out_blk, F4)
        nc.sync.dma_start(
            out_o[: 97 * 128, :].rearrange("(m q) f -> q m f", q=128),
            ob[:, :97, :3])
        nc.sync.dma_start(out_o[97 * 128:NLOC, :], ob[:84, 97, :3])

    _split_waits(nc)
    return nc


_CACHE = {}


def kernel(x, edge_index, laplacian, W1, b1, W2, b2, W3, b3, W4, b4):
    ins, G = _host_prep(x, edge_index, laplacian)
    wpack, offs = _weights_prep(W1, W2, W3, W4, b1, b2, b3, b4)
    use_shift = all(i["all_nonzero"] for i in ins)
    key = (G, use_shift)
    if key not in _CACHE:
        _CACHE[key] = build_program(G, offs, wpack.shape[1], use_shift)
    nc = _CACHE[key]
    in_maps = []
    for p in range(NC):
        m = dict(colidx=ins[p]["colidx"], nrm=ins[p]["nrm"],
                 ends=ins[p]["ends"], starts=ins[p]["starts"],
                 csmask=ins[p]["csmask"], xblk=ins[p]["xblk"], wpack=wpack)
        in_maps.append(m)
    res = run_bass_kernel_spmd(nc, in_maps, core_ids=list(range(NC)))
    out = np.concatenate([r["out"] for r in res.results], axis=0)
    return np.ascontiguousarray(out, dtype=np.float32)



# revision 2
# speedup vs baseline: 1.0924x; 1.0924x over previous
"""DiffusionNetAutoencoder (4x ChebConv, K=6) Trainium2 kernel on 8 NeuronCores.

v2: nodes dest-sharded across 8 cores; per core edges sorted by dest, packed
into 128-slot chunks with NO padding (prefix sampled at every slot via a full
lower-triangular QTRI matmul). Per-dest sums = prefix[end] - prefix[prev end]
(cs-shift). qbuf is split into overlapping piece tensors (14 dest-chunks per
piece) so ends-gathers / combine / block writes stream behind the gather
pipeline instead of serializing after it.

Tables are laid out [8 * NLOC_PAD, f] (padded-chunk-major per core) so block
writes and AllGathers are uniform.
"""
import contextlib
import numpy as np

import bass_rust
import concourse.bass as bass
import concourse.tile as tile
import concourse.mybir as mybir
from concourse.bass_utils import run_bass_kernel_spmd

F32 = mybir.dt.float32
I32 = mybir.dt.int32
AO = mybir.AluOpType

N_NODES = 100000
K = 6
NC = 8
NLOC = N_NODES // NC          # 12500
NLOC_PAD = 12544              # 128 * 98
NCH = NLOC_PAD // 128         # 98 chunks of local dests
NTAB = NC * NLOC_PAD          # table rows
F_HID, F_LAT, F4 = 128, 32, 4
PCH = 14                      # dest-chunks per piece
NP = NCH // PCH               # 7 pieces


# --------------------------------------------------------------------------
# This container's walrus rejects >1 sync-wait per instruction; Tile attaches
# more. Split extras onto NoOp carriers (same engine, just before the inst).
def _split_waits(nc, max_waits=1):
    for f in nc.m.functions:
        for bb in f.blocks:
            new_insts = []
            for inst in bb.instructions:
                si = inst.sync_info
                if si is not None and si.on_wait and len(si.on_wait) > max_waits:
                    waits = list(si.on_wait)
                    extra, keep = waits[:-max_waits], waits[-max_waits:]
                    si.on_wait = keep
                    for j in range(0, len(extra), max_waits):
                        nop = bass_rust.InstNoOp(
                            name=nc.get_next_instruction_name(),
                            opcode="NoOp",
                            engine=inst.engine,
                            debug=inst.debug,
                            ins=[], outs=[],
                            descendants=bass_rust.InstructionNameOrderedSet(),
                            sync_info=mybir.SyncInfo(
                                on_wait=extra[j:j + max_waits], on_update=[]),
                            bass_nofuse=True,
                            text_hint=None,
                        )
                        nc.register_instruction(nop, overwrite=True)
                        new_insts.append(nop)
                new_insts.append(inst)
            bb.instructions[:] = new_insts


def _piece_ranges(G):
    """Static group ranges per piece: piece p serves dest-chunks
    [p*PCH, (p+1)*PCH); its qbuf tensor holds groups [gs, ge)."""
    rng = []
    for p in range(NP):
        gs = max(0, G * (p * PCH) // NCH - 2)
        ge = min(G, G * ((p + 1) * PCH) // NCH + 3)
        rng.append((gs, ge))
    rng[-1] = (rng[-1][0], G)
    return rng


# --------------------------------------------------------------------------
def _host_prep(x, edge_index, laplacian):
    row = np.asarray(edge_index[0], np.int64)
    col = np.asarray(edge_index[1], np.int64)
    lap = np.asarray(laplacian, np.float32)

    w = np.where(row == col, np.float32(0), lap).astype(np.float32)
    deg = np.zeros(N_NODES, np.float32)
    np.add.at(deg, row, w)
    dis = np.where(deg > 0, 1.0 / np.sqrt(np.maximum(deg, np.float32(1e-30))), 0.0)
    dis = dis.astype(np.float32)
    norm = (-dis[row] * w * dis[col]).astype(np.float32)

    # remap cols to padded table rows
    colp = (col // NLOC) * NLOC_PAD + (col % NLOC)

    owner = row // NLOC
    per_core = []
    for p in range(NC):
        sel = np.nonzero(owner == p)[0]
        r_loc = (row[sel] - p * NLOC).astype(np.int64)
        order = np.argsort(r_loc, kind="stable")
        sel = sel[order]
        r_loc = r_loc[order]
        c_e = colp[sel].astype(np.int32)
        n_e = norm[sel]

        d = np.bincount(r_loc, minlength=NLOC).astype(np.int64)
        assert d.max() <= 128, f"in-degree too large for one chunk: {d.max()}"

        seg_start = np.zeros(NLOC, np.int64)
        pos = 0
        for i in range(NLOC):
            di = d[i]
            if di == 0:
                seg_start[i] = pos
                continue
            room = 128 - (pos & 127)
            if di > room:
                pos += room
            seg_start[i] = pos
            pos += di

        first_edge = np.zeros(NLOC, np.int64)
        first_edge[1:] = np.cumsum(d)[:-1]
        rank = np.arange(len(sel), dtype=np.int64) - first_edge[r_loc]
        slot = seg_start[r_loc] + rank
        per_core.append(dict(c_e=c_e, n_e=n_e, slot=slot, d=d,
                             seg_start=seg_start, e_used=pos))

    e_pad = max(pc["e_used"] for pc in per_core)
    G = (e_pad + 2047) // 2048
    e_pad = G * 2048
    pr = _piece_ranges(G)
    prows = [(ge - gs) * 2048 for gs, ge in pr]

    ins = []
    for p in range(NC):
        pc = per_core[p]
        cols = np.zeros(e_pad, np.int32)
        nrms = np.zeros(e_pad, np.float32)
        cols[pc["slot"]] = pc["c_e"]
        nrms[pc["slot"]] = pc["n_e"]
        colt = cols.reshape(G * 16, 128).T.copy()
        nrmt = nrms.reshape(G * 16, 128).T.copy()

        d, seg_start = pc["d"], pc["seg_start"]
        has = d > 0
        e_slot = seg_start + d - 1          # last slot of each run (deg>0)
        # piece-local ends / starts offsets
        ends = np.zeros(NLOC_PAD, np.int64)
        starts = np.zeros(NLOC_PAD, np.int64)
        msk = np.zeros(NLOC_PAD, np.float32)
        ok = True
        for q in range(NP):
            lo, hi = q * PCH * 128, min((q + 1) * PCH * 128, NLOC)
            if lo >= NLOC:
                ends[lo:] = prows[q]
                starts[lo:] = prows[q]
                continue
            gs, ge = pr[q]
            zq = prows[q]
            dl = d[lo:hi]
            hl = has[lo:hi]
            el = e_slot[lo:hi] - gs * 2048
            sl = seg_start[lo:hi] - 1 - gs * 2048
            if hl.any():
                if el[hl].min() < 0 or el[hl].max() >= zq:
                    ok = False
            e_l = np.where(hl, el, zq)
            s_ok = hl & ((seg_start[lo:hi] & 127) != 0)
            if s_ok.any():
                if sl[s_ok].min() < 0 or sl[s_ok].max() >= zq:
                    ok = False
            s_l = np.where(s_ok, sl, zq)
            ends[lo:hi] = e_l
            starts[lo:hi] = s_l
            if hi < (q + 1) * PCH * 128:
                ends[hi:(q + 1) * PCH * 128] = zq
                starts[hi:(q + 1) * PCH * 128] = zq
            msk[lo:hi][s_ok] = 1.0
        endt = ends.reshape(NCH, 128).T.astype(np.int32).copy()
        stat = starts.reshape(NCH, 128).T.astype(np.int32).copy()
        mskt = msk.reshape(NCH, 128).T.copy()

        xb = np.zeros((NLOC_PAD, F4), np.float32)
        xb[:NLOC, :3] = np.asarray(x[p * NLOC:(p + 1) * NLOC], np.float32)
        ins.append(dict(colidx=colt, nrm=nrmt, ends=endt, starts=stat,
                        xblk=xb, csmask=mskt,
                        all_nonzero=bool((d > 0).all()), pieces_ok=ok))
    return ins, G


def _weights_prep(W1, W2, W3, W4, b1, b2, b3, b4):
    W1 = np.asarray(W1, np.float32); W2 = np.asarray(W2, np.float32)
    W3 = np.asarray(W3, np.float32); W4 = np.asarray(W4, np.float32)
    w1s = np.zeros((128, F_HID), np.float32)
    for k in range(K):
        w1s[k * 4:k * 4 + 3, :] = W1[k]
    w2s = W2.transpose(1, 0, 2).reshape(F_HID, K * F_LAT)
    w3hi = W3[:4].reshape(4 * F_LAT, F_HID)
    w3lo = np.zeros((128, F_HID), np.float32)
    w3lo[:64] = W3[4:].reshape(2 * F_LAT, F_HID)
    w4s = np.zeros((F_HID, K * F4), np.float32)
    for k in range(K):
        w4s[:, k * 4:k * 4 + 3] = W4[k]
    qt1 = np.tril(np.ones((128, 128), np.float32)).T.copy()  # qt1[i,j]=1, i<=j
    qt2 = 2.0 * qt1
    ident = np.eye(128, dtype=np.float32)
    bc1 = np.zeros((128, 1), np.float32); bc1[:, 0] = np.asarray(b1, np.float32)
    br2 = np.zeros((128, F_LAT), np.float32); br2[:] = np.asarray(b2, np.float32)
    bc3 = np.zeros((128, 1), np.float32); bc3[:, 0] = np.asarray(b3, np.float32)
    b4p = np.zeros(F4, np.float32); b4p[:3] = np.asarray(b4, np.float32)
    br4 = np.zeros((128, F4), np.float32); br4[:] = b4p
    parts = [("w1s", w1s), ("w2sA", w2s[:, :128]), ("w2sB", w2s[:, 128:]),
             ("w3hi", w3hi), ("w3lo", w3lo), ("w4s", w4s), ("qt1", qt1),
             ("qt2", qt2), ("ident", ident), ("bc1", bc1), ("br2", br2),
             ("bc3", bc3), ("br4", br4)]
    offs, cur = {}, 0
    for nm, a in parts:
        assert a.shape[0] <= 128
        offs[nm] = (cur, a.shape[0], a.shape[1])
        cur += a.shape[1]
    wpack = np.zeros((128, cur), np.float32)
    for nm, a in parts:
        c0, r, c = offs[nm]
        wpack[:r, c0:c0 + c] = a
    return wpack, offs


# --------------------------------------------------------------------------
def build_program(G, offs, wtot, use_shift=False):
    EP = G * 2048
    pr = _piece_ranges(G)
    prows = [(ge - gs) * 2048 for gs, ge in pr]
    nc = bass.Bass(trn_type="TRN2")

    colidx_i = nc.dram_tensor("colidx", [128, G * 16], I32, kind="ExternalInput")
    nrm_i = nc.dram_tensor("nrm", [128, G * 16], F32, kind="ExternalInput")
    ends_i = nc.dram_tensor("ends", [128, NCH], I32, kind="ExternalInput")
    starts_i = nc.dram_tensor("starts", [128, NCH], I32, kind="ExternalInput")
    csmask_i = nc.dram_tensor("csmask", [128, NCH], F32, kind="ExternalInput")
    xblk_i = nc.dram_tensor("xblk", [NLOC_PAD, F4], F32, kind="ExternalInput")
    wpack_i = nc.dram_tensor("wpack", [128, wtot], F32, kind="ExternalInput")
    out_o = nc.dram_tensor("out", [NLOC, 3], F32, kind="ExternalOutput")

    t4a = nc.dram_tensor("t4a", [NTAB, F4], F32, addr_space="Shared")
    t4b = nc.dram_tensor("t4b", [NTAB, F4], F32, addr_space="Shared")
    t32a = nc.dram_tensor("t32a", [NTAB, F_LAT], F32, addr_space="Shared")
    t32b = nc.dram_tensor("t32b", [NTAB, F_LAT], F32, addr_space="Shared")
    q4 = [nc.dram_tensor(f"q4_{p}", [prows[p] + 1, F4], F32) for p in range(NP)]
    q32 = [nc.dram_tensor(f"q32_{p}", [prows[p] + 1, F_LAT], F32)
           for p in range(NP)]
    blk4 = nc.dram_tensor("blk4", [NLOC_PAD, F4], F32)
    blk32 = nc.dram_tensor("blk32", [NLOC_PAD, F_LAT], F32)
    ybuf2 = nc.dram_tensor("ybuf2", [NLOC_PAD, K * F_LAT], F32)
    ybuf4 = nc.dram_tensor("ybuf4", [NLOC_PAD, K * F4], F32)
    h1T_d = nc.dram_tensor("h1T_d", [F_HID, NLOC_PAD], F32)
    h3T_d = nc.dram_tensor("h3T_d", [F_HID, NLOC_PAD], F32)
    thi_d = nc.dram_tensor("thi_d", [128, NLOC_PAD], F32)
    tlo_d = nc.dram_tensor("tlo_d", [64, NLOC_PAD], F32)

    RG = [list(range(NC))]

    with tile.TileContext(nc) as tc, contextlib.ExitStack() as ctx:
        sbc = ctx.enter_context(tc.tile_pool(name="sbc", bufs=1))
        big = ctx.enter_context(tc.tile_pool(name="big", bufs=1))
        gb = ctx.enter_context(tc.tile_pool(name="gb", bufs=3))
        ykp = ctx.enter_context(tc.tile_pool(name="ykp", bufs=2))
        bd = ctx.enter_context(tc.tile_pool(name="bd", bufs=1))
        dtp = ctx.enter_context(tc.tile_pool(name="dtp", bufs=2))
        kpp = ctx.enter_context(tc.tile_pool(name="kpp", bufs=2))
        st = ctx.enter_context(tc.tile_pool(name="st", bufs=2))
        ps = ctx.enter_context(tc.tile_pool(name="ps", bufs=4, space="PSUM"))
        ps2 = ctx.enter_context(tc.tile_pool(name="ps2", bufs=2, space="PSUM"))

        # ---- constants
        colidx = sbc.tile([128, G * 16], I32)
        nc.sync.dma_start(colidx[:], colidx_i[:])
        nrm1 = sbc.tile([128, G * 16], F32)
        nc.sync.dma_start(nrm1[:], nrm_i[:])
        ends_sb = sbc.tile([128, NCH], I32)
        nc.sync.dma_start(ends_sb[:], ends_i[:])
        starts_sb = sbc.tile([128, NCH], I32)
        nc.sync.dma_start(starts_sb[:], starts_i[:])
        csmask = sbc.tile([128, NCH], F32)
        nc.sync.dma_start(csmask[:], csmask_i[:])
        wpack = sbc.tile([128, wtot], F32)
        nc.sync.dma_start(wpack[:], wpack_i[:])
        wt = {nm: wpack[:r, c0:c0 + c] for nm, (c0, r, c) in offs.items()}

        zt = st.tile([128, F_LAT], F32, tag="zt")
        nc.vector.memset(zt[:], 0.0)
        for p in range(NP):
            nc.sync.dma_start(q4[p][prows[p]:prows[p] + 1, :], zt[:1, :F4])
            nc.sync.dma_start(q32[p][prows[p]:prows[p] + 1, :], zt[:1, :F_LAT])

        def r3(t, f):
            return t[:].rearrange("p (m f) -> p m f", f=f)

        def blk_piece(dt_ap, dram, f, q):
            lo = q * PCH
            nc.sync.dma_start(
                dram[lo * 128:(lo + PCH) * 128, :].rearrange(
                    "(m q) f -> q m f", q=128),
                dt_ap.rearrange("p (m f) -> p m f", f=f)[:, lo:lo + PCH, :])

        def ag(blk, table):
            nc.gpsimd.collective_compute(
                "AllGather", AO.bypass, replica_groups=RG,
                ins=[blk.ap().opt()], outs=[table.ap().opt()])

        # x block -> x table
        x_loc = big.tile([128, NCH * F4], F32, tag="xloc")
        nc.sync.dma_start(
            r3(x_loc, F4),
            xblk_i[:].rearrange("(m q) f -> q m f", q=128))
        for q in range(NP):
            blk_piece(x_loc[:], blk4, F4, q)
        ag(blk4, t4a)

        # ---- spmm: gathers stream per piece; combine(dt, q) is called per
        # piece after its dt slice (ce-cs) is computed; returns whole dt tile.
        def spmm(src_table, qbuf, f, scale2, blkdram, combine=None,
                 last_ag=None):
            qtri = wt["qt2"] if scale2 else wt["qt1"]
            ce = bd.tile([128, NCH * 32], F32, tag="ce")
            cs = bd.tile([128, NCH * 32], F32, tag="cs")
            dt = dtp.tile([128, NCH * 32], F32, tag="dt")
            gdone = set()
            for q in range(NP):
                gs, ge = pr[q]
                for g in range(gs, ge):
                    if g in gdone:
                        continue
                    gdone.add(g)
                    gt = gb.tile([128, 16 * f], F32, tag="gt")
                    for m in range(16):
                        nc.gpsimd.indirect_dma_start(
                            out=gt[:, m * f:(m + 1) * f], out_offset=None,
                            in_=src_table[:],
                            in_offset=bass.IndirectOffsetOnAxis(
                                ap=colidx[:, g * 16 + m:g * 16 + m + 1],
                                axis=0))
                    vt = gb.tile([128, 16 * f], F32, tag="vt")
                    nc.vector.tensor_tensor(
                        out=r3(vt, f), in0=r3(gt, f),
                        in1=nrm1[:, g * 16:(g + 1) * 16][:, :, None]
                        .to_broadcast([128, 16, f]),
                        op=AO.mult)
                    pst = ps.tile([128, 16 * f], F32, tag="mm")
                    nc.tensor.matmul(pst[:], lhsT=qtri, rhs=vt[:],
                                     start=True, stop=True)
                    qs = gb.tile([128, 16 * f], F32, tag="qs")
                    nc.vector.tensor_copy(out=qs[:], in_=pst[:])
                    for q2 in range(NP):
                        g2s, g2e = pr[q2]
                        if g2s <= g < g2e:
                            nc.sync.dma_start(
                                qbuf[q2][(g - g2s) * 2048:
                                         (g - g2s + 1) * 2048, :].rearrange(
                                    "(m j) f -> j m f", j=128),
                                r3(qs, f))
                # ends/starts gathers for this piece's chunks
                for c in range(q * PCH, (q + 1) * PCH):
                    nc.gpsimd.indirect_dma_start(
                        out=ce[:, c * f:(c + 1) * f], out_offset=None,
                        in_=qbuf[q][:],
                        in_offset=bass.IndirectOffsetOnAxis(
                            ap=ends_sb[:, c:c + 1], axis=0))
                    if not use_shift:
                        nc.gpsimd.indirect_dma_start(
                            out=cs[:, c * f:(c + 1) * f], out_offset=None,
                            in_=qbuf[q][:],
                            in_offset=bass.IndirectOffsetOnAxis(
                                ap=starts_sb[:, c:c + 1], axis=0))
                lo, hi = q * PCH, (q + 1) * PCH
                if use_shift:
                    # cs[r, c] = ce[r-1, c]; cs[0, c] = ce[127, c-1]; masked
                    nc.scalar.dma_start(cs[1:128, lo * f:hi * f],
                                        ce[0:127, lo * f:hi * f])
                    if lo == 0:
                        nc.vector.memset(cs[0:1, :f], 0.0)
                        if hi > 1:
                            nc.scalar.dma_start(cs[0:1, f:hi * f],
                                                ce[127:128, :(hi - 1) * f])
                    else:
                        nc.scalar.dma_start(cs[0:1, lo * f:hi * f],
                                            ce[127:128, (lo - 1) * f:(hi - 1) * f])
                    nc.vector.tensor_tensor(
                        out=cs[:, lo * f:hi * f].rearrange(
                            "p (m f) -> p m f", f=f),
                        in0=cs[:, lo * f:hi * f].rearrange(
                            "p (m f) -> p m f", f=f),
                        in1=csmask[:, lo:hi, None].to_broadcast([128, PCH, f]),
                        op=AO.mult)
                nc.vector.tensor_tensor(out=dt[:, lo * f:hi * f],
                                        in0=ce[:, lo * f:hi * f],
                                        in1=cs[:, lo * f:hi * f],
                                        op=AO.subtract)
                if combine is not None:
                    combine(dt, q)
                if blkdram is not None:
                    blk_piece(dt[:, :NCH * f], blkdram, f, q)
            if last_ag is not None:
                ag(blkdram, last_ag)
            return dt

        def relu(ap):
            nc.vector.tensor_scalar_max(ap, ap, 0.0)

        def dcut(dt, f):
            return dt[:, : NCH * f]

        def rd3(dt, f):
            return dt[:, : NCH * f].rearrange("p (m f) -> p m f", f=f)

        def pslice(dt, f, q):
            return dt[:, q * PCH * f:(q + 1) * PCH * f]

        def p3(ap, f, q):
            return ap[:, q * PCH * f:(q + 1) * PCH * f].rearrange(
                "p (m f) -> p m f", f=f)

        # ============== Layer 1: forward, width 4, input x ==============
        txrm = big.tile([128, NCH * K * F4], F32, tag="txrm")

        def txrm_k(kk):
            return txrm[:].rearrange("p (m k f) -> p m k f", k=K, f=F4)[:, :, kk, :]

        def txrm_kp(kk, q):
            return txrm[:].rearrange(
                "p (m k f) -> p m k f", k=K, f=F4)[:, q * PCH:(q + 1) * PCH, kk, :]

        nc.vector.tensor_copy(out=txrm_k(0), in_=r3(x_loc, F4))
        tabs = [t4a, t4b]
        for k in range(1, K):
            kk = k

            def comb1(dt, q, kk=kk):
                if kk > 1:
                    nc.vector.tensor_tensor(
                        out=p3(dt, F4, q), in0=p3(dt, F4, q),
                        in1=txrm_kp(kk - 2, q), op=AO.subtract)
                nc.vector.tensor_copy(out=txrm_kp(kk, q), in_=p3(dt, F4, q))

            spmm(tabs[(k + 1) % 2], q4, F4, scale2=(k > 1),
                 blkdram=blk4 if k < K - 1 else None, combine=comb1,
                 last_ag=tabs[k % 2] if k < K - 1 else None)

        # dense L1: h1T = relu(w1s.T @ TxT + b1), staged to DRAM
        for c0 in range(0, NCH, 4):
            cn = min(4, NCH - c0)
            ho = st.tile([128, 4 * 128], F32, tag="ho")
            for c in range(c0, c0 + cn):
                ptr = ps2.tile([24, 128], F32, tag="tp")
                nc.tensor.transpose(ptr[:], txrm[:, c * 24:(c + 1) * 24],
                                    wt["ident"])
                t1t = st.tile([24, 128], F32, tag="sa")
                nc.vector.tensor_copy(out=t1t[:], in_=ptr[:])
                po = ps.tile([128, 128], F32, tag="mm")
                nc.tensor.matmul(po[:], lhsT=wt["w1s"][:24, :], rhs=t1t[:],
                                 start=True, stop=True)
                nc.vector.tensor_tensor(
                    out=ho[:, (c - c0) * 128:(c - c0 + 1) * 128], in0=po[:],
                    in1=wt["bc1"].to_broadcast([128, 128]), op=AO.add)
            relu(ho[:, : cn * 128])
            nc.sync.dma_start(h1T_d[:, c0 * 128:(c0 + cn) * 128], ho[:, : cn * 128])

        # ============== Layer 2: Clenshaw, width 32 ==============
        for c0 in range(0, NCH, 4):
            cn = min(4, NCH - c0)
            yst = st.tile([128, 4 * K * F_LAT], F32, tag="yst")
            for c in range(c0, c0 + cn):
                ht = st.tile([128, 128], F32, tag="ht")
                nc.scalar.dma_start(ht[:], h1T_d[:, c * 128:(c + 1) * 128])
                pa = ps.tile([128, 128], F32, tag="mm")
                nc.tensor.matmul(pa[:], lhsT=wt["w2sA"], rhs=ht[:],
                                 start=True, stop=True)
                ya = st.tile([128, 128], F32, tag="sa")
                nc.vector.tensor_copy(out=ya[:], in_=pa[:])
                pb = ps.tile([64, 128], F32, tag="mm")
                nc.tensor.matmul(pb[:], lhsT=wt["w2sB"], rhs=ht[:],
                                 start=True, stop=True)
                yb = st.tile([64, 128], F32, tag="sa")
                nc.vector.tensor_copy(out=yb[:], in_=pb[:])
                pt = ps2.tile([128, 128], F32, tag="tp")
                nc.tensor.transpose(pt[:], ya[:], wt["ident"])
                nc.vector.tensor_copy(
                    out=yst[:, (c - c0) * 192:(c - c0) * 192 + 128], in_=pt[:])
                pt2 = ps2.tile([128, 64], F32, tag="tp")
                nc.tensor.transpose(pt2[:], yb[:], wt["ident"][:64, :64])
                nc.vector.tensor_copy(
                    out=yst[:, (c - c0) * 192 + 128:(c - c0 + 1) * 192], in_=pt2[:])
            nc.sync.dma_start(
                ybuf2[c0 * 128:(c0 + cn) * 128, :].rearrange(
                    "(m q) f -> q m f", q=128),
                yst[:].rearrange("p (m f) -> p m f", f=K * F_LAT)[:, :cn, :])

        def clenshaw2(f, ybuf, tabs2, qbuf, blk, bias_row, last_to):
            ydram = ybuf.ap().rearrange("n (k f) -> n k f", f=f)

            def yk_load(kk):
                t = ykp.tile([128, NCH * 32], F32, tag="yk")
                nc.scalar.dma_start(
                    t[:, : NCH * f].rearrange("p (m f) -> p m f", f=f),
                    ydram[:, kk, :].rearrange("(m q) f -> q m f", q=128))
                return t

            y5 = yk_load(K - 1)
            k5 = kpp.tile([128, NCH * 32], F32, tag="kc")
            nc.vector.tensor_copy(out=k5[:, : NCH * f], in_=y5[:, : NCH * f])
            for q in range(NP):
                blk_piece(k5[:, :NCH * f], blk, f, q)
            ag(blk, tabs2[0])
            bk2_loc, bk1_loc = None, k5
            cur = 0
            out_dt = None
            for k in range(K - 2, -1, -1):
                yk = yk_load(k)
                if bk2_loc is not None:
                    nc.vector.tensor_tensor(out=yk[:, : NCH * f],
                                            in0=yk[:, : NCH * f],
                                            in1=bk2_loc[:, : NCH * f],
                                            op=AO.subtract)
                if k == 0:
                    kc = None

                    def comb0(dt, q, yk=yk):
                        nc.vector.tensor_tensor(
                            out=pslice(dt, f, q), in0=pslice(dt, f, q),
                            in1=pslice(yk, f, q), op=AO.add)
                        nc.vector.tensor_tensor(
                            out=p3(dt, f, q), in0=p3(dt, f, q),
                            in1=bias_row[:, None, :].to_broadcast(
                                [128, PCH, f]),
                            op=AO.add)
                        relu(pslice(dt, f, q))

                    out_dt = spmm(tabs2[cur], qbuf, f, scale2=False,
                                  blkdram=blk if last_to is not None else None,
                                  combine=comb0, last_ag=last_to)
                    return out_dt
                kc = kpp.tile([128, NCH * 32], F32, tag="kc")

                def combk(dt, q, yk=yk, kc=kc):
                    nc.vector.tensor_tensor(
                        out=pslice(dt, f, q), in0=pslice(dt, f, q),
                        in1=pslice(yk, f, q), op=AO.add)
                    nc.vector.tensor_copy(out=pslice(kc, f, q),
                                          in_=pslice(dt, f, q))

                spmm(tabs2[cur], qbuf, f, scale2=(k > 0), blkdram=blk,
                     combine=combk, last_ag=tabs2[1 - cur])
                bk2_loc, bk1_loc = bk1_loc, kc
                cur = 1 - cur

        h2 = clenshaw2(F_LAT, ybuf2, [t32a, t32b], q32, blk32, wt["br2"],
                       last_to=t32a)

        # ============== Layer 3: forward, width 32 ==============
        def tx_to_T(src_ap, kk, c0, cn):
            dst = thi_d if kk < 4 else tlo_d
            r0 = (kk % 4) * 32 if kk < 4 else (kk - 4) * 32
            for cb in range(c0, c0 + cn, 4):
                cbn = min(4, c0 + cn - cb)
                ts = st.tile([32, 4 * 128], F32, tag="ts")
                for c in range(cb, cb + cbn):
                    pt = ps2.tile([32, 128], F32, tag="tp")
                    nc.tensor.transpose(pt[:], src_ap[:, c * 32:(c + 1) * 32],
                                        wt["ident"])
                    nc.vector.tensor_copy(
                        out=ts[:, (c - cb) * 128:(c - cb + 1) * 128], in_=pt[:])
                nc.sync.dma_start(dst[r0:r0 + 32, cb * 128:(cb + cbn) * 128],
                                  ts[:, : cbn * 128])

        tx_to_T(dcut(h2, F_LAT), 0, 0, NCH)
        h2k = kpp.tile([128, NCH * 32], F32, tag="kc")
        nc.vector.tensor_copy(out=h2k[:], in_=h2[:])
        prev2, prev1 = None, h2k
        tabs3 = [t32a, t32b]
        cur = 0
        for k in range(1, K):
            kk = k
            kc = kpp.tile([128, NCH * 32], F32, tag="kc")
            pv2 = prev2

            def comb3(dt, q, kk=kk, pv2=pv2, kc=kc):
                if kk > 1:
                    nc.vector.tensor_tensor(
                        out=pslice(dt, F_LAT, q), in0=pslice(dt, F_LAT, q),
                        in1=pslice(pv2, F_LAT, q), op=AO.subtract)
                tx_to_T(dcut(dt, F_LAT), kk, q * PCH, PCH)
                nc.vector.tensor_copy(out=pslice(kc, F_LAT, q),
                                      in_=pslice(dt, F_LAT, q))

            spmm(tabs3[cur], q32, F_LAT, scale2=(k > 1),
                 blkdram=blk32 if k < K - 1 else None, combine=comb3,
                 last_ag=tabs3[1 - cur] if k < K - 1 else None)
            if k < K - 1:
                cur = 1 - cur
            prev2, prev1 = prev1, kc

        # dense L3: h3T = relu(w3hi.T @ thi + w3lo.T @ tlo + b3)
        for c0 in range(0, NCH, 4):
            cn = min(4, NCH - c0)
            ho = st.tile([128, 4 * 128], F32, tag="ho")
            for c in range(c0, c0 + cn):
                ti = st.tile([128, 128], F32, tag="ht")
                nc.scalar.dma_start(ti[:], thi_d[:, c * 128:(c + 1) * 128])
                tl = st.tile([64, 128], F32, tag="tl")
                nc.scalar.dma_start(tl[:], tlo_d[:64, c * 128:(c + 1) * 128])
                po = ps.tile([128, 128], F32, tag="mm")
                nc.tensor.matmul(po[:], lhsT=wt["w3hi"], rhs=ti[:],
                                 start=True, stop=False)
                nc.tensor.matmul(po[:], lhsT=wt["w3lo"][:64, :], rhs=tl[:],
                                 start=False, stop=True)
                nc.vector.tensor_tensor(
                    out=ho[:, (c - c0) * 128:(c - c0 + 1) * 128], in0=po[:],
                    in1=wt["bc3"].to_broadcast([128, 128]), op=AO.add)
            relu(ho[:, : cn * 128])
            nc.sync.dma_start(h3T_d[:, c0 * 128:(c0 + cn) * 128], ho[:, : cn * 128])

        # ============== Layer 4: Clenshaw, width 4 ==============
        for c0 in range(0, NCH, 4):
            cn = min(4, NCH - c0)
            yst = st.tile([128, 4 * K * F4], F32, tag="yst4")
            for c in range(c0, c0 + cn):
                ht = st.tile([128, 128], F32, tag="ht")
                nc.scalar.dma_start(ht[:], h3T_d[:, c * 128:(c + 1) * 128])
                pa = ps.tile([K * F4, 128], F32, tag="mm")
                nc.tensor.matmul(pa[:], lhsT=wt["w4s"], rhs=ht[:],
                                 start=True, stop=True)
                ya = st.tile([K * F4, 128], F32, tag="sa")
                nc.vector.tensor_copy(out=ya[:], in_=pa[:])
                pt = ps2.tile([128, K * F4], F32, tag="tp")
                nc.tensor.transpose(pt[:], ya[:], wt["ident"][: K * F4, : K * F4])
                nc.vector.tensor_copy(
                    out=yst[:, (c - c0) * K * F4:(c - c0 + 1) * K * F4], in_=pt[:])
            nc.sync.dma_start(
                ybuf4[c0 * 128:(c0 + cn) * 128, :].rearrange(
                    "(m q) f -> q m f", q=128),
                yst[:].rearrange("p (m f) -> p m f", f=K * F4)[:, :cn, :])

        out_blk = clenshaw2(F4, ybuf4, [t4a, t4b], q4, blk4, wt["br4"],
                            last_to=None)
        ob = rd3(out_blk, F4)
        nc.sync.dma_start(
            out_o[: 97 * 128, :].rearrange("(m q) f -> q m f", q=128),
            ob[:, :97, :3])
        nc.sync.dma_start(out_o[97 * 128:NLOC, :], ob[:84, 97, :3])

    _split_waits(nc)
    return nc


_CACHE = {}


def kernel(x, edge_index, laplacian, W1, b1, W2, b2, W3, b3, W4, b4):
    ins, G = _host_prep(x, edge_index, laplacian)
    wpack, offs = _weights_prep(W1, W2, W3, W4, b1, b2, b3, b4)
    use_shift = all(i["all_nonzero"] for i in ins)
    assert all(i["pieces_ok"] for i in ins), "piece window overflow"
    key = (G, use_shift)
    if key not in _CACHE:
        _CACHE[key] = build_program(G, offs, wpack.shape[1], use_shift)
    nc = _CACHE[key]
    in_maps = []
    for p in range(NC):
        m = dict(colidx=ins[p]["colidx"], nrm=ins[p]["nrm"],
                 ends=ins[p]["ends"], starts=ins[p]["starts"],
                 csmask=ins[p]["csmask"], xblk=ins[p]["xblk"], wpack=wpack)
        in_maps.append(m)
    res = run_bass_kernel_spmd(nc, in_maps, core_ids=list(range(NC)))
    out = np.concatenate([r["out"] for r in res.results], axis=0)
    return np.ascontiguousarray(out, dtype=np.float32)
